# revision 39
# baseline (speedup 1.0000x reference)
"""Two-layer GAT (PyG-style, eval mode) on 8 Trainium2 NeuronCores.

Sharding: dst nodes are processed in 128-node tiles; tiles are LPT-assigned
to cores to balance padded edge-group counts (the SPMD schedule is the
per-slot max over cores, so each core also sorts its tiles by size).  The
node->table-row permutation this induces is applied host-side to the gather
indices and inverted on the output.

Design (v2, instruction-count-driven):
- Layer-1 aggregates in h1-space: a per-node table row holds
  [h1 (8 heads x 64) | a_src (8) | pad] bf16 (640 cols = 1280 B, the 256 B
  gather granularity), built per local slice then AllGathered in two halves
  (each half overlaps other work).  One dma_gather per 4 (layer 1) / 8
  (layer 2) edge groups; a_src rides the table row, so the baseline's
  transposed per-edge gather is gone.
- One-hot selection matrices S ([edge, slot]) and their transposes ST
  ([slot, edge]) are host-precomputed in fp8e4 and DMA-shipped per tile:
  no per-group is_equal builds and no per-group PE transposes.  ST slices
  feed per-group a_dst matmuls (lhsT=ST_j, rhs=a_dst_node) into one PSUM
  strip for the whole tile, and an identity-lhsT matmul accumulates the
  gathered a_src on top; S slices are the lhsT of the segment-sum (z) and
  softmax-denominator (s) matmuls.
- Per-edge softmax weights for a whole tile: one DVE leaky-relu (reading
  the logits straight from PSUM) and one ScalarE exp.
- The message product M = h1_gathered * p uses a duplicated-pair access
  pattern (innermost [2]-stride-1 on every operand) to hit the DVE 2x
  16-bit mode.
- Layer-2 table rows: [h2 (64) | 1.0 | a_src2 | pad] (256 B).  The ones
  column makes the z matmul produce the softmax denominator for free.
segment-max is skipped: logits are O(+-8) so exp() is safe in fp32, and
softmax is shift-invariant so the result matches the reference.
"""

import os
from contextlib import ExitStack

import numpy as np

# ----------------------------------------------------------------------------
# problem config (hardcoded per contest contract)
# ----------------------------------------------------------------------------
CFG = dict(
    N=50000,       # nodes
    IN=128,        # input feature dim
    HID=64,        # per-head hidden dim
    H1=8,          # layer-1 heads
    NCORES=8,
)

P = 128    # partitions / tile edge
GCH = 4    # edge groups per dma_gather chunk, layer 1
GCH2 = 8   # edge groups per dma_gather chunk, layer 2 (1024 rows)
MCH = 8    # edge groups per message-product chunk
TW1 = 640  # layer-1 table row cols (bf16): 512 h1 + 8 a_src + pad
TW2 = 128  # layer-2 table row cols: 64 h2 + 1 ones + 1 a_src2 + pad
NT = 49    # dst tiles per core (8*49*128 = 50176 >= 50000)
KSPLIT = 25  # tiles per core in table half A (AllGather split point)


def _cdiv(a, b):
    return (a + b - 1) // b


# ----------------------------------------------------------------------------
# host-side sharding prep (pure layout work: sort, bucket, pad, pack indices)
# ----------------------------------------------------------------------------
def prep_edges(edge_index, cfg):
    """Sort self-loop-augmented edges by dst; LPT-assign 128-dst-node tiles
    to cores balancing padded group counts; build per-core packed gather
    indices (lo/hi table halves for the int16 limit), fp8 one-hot S/ST
    selection matrices, the node->table-row permutation, and the softmax
    pad-row masks.
    """
    import ml_dtypes

    N, NC = cfg["N"], cfg["NCORES"]
    NPCP = NT * P                  # padded rows per core (6272)
    NTAB = NC * NPCP               # permuted table rows (50176)
    HALF = NC * KSPLIT * P         # rows in table half A (= lo/hi boundary)
    NTILES = _cdiv(N, P)           # 391 global tiles

    src = np.concatenate([edge_index[0].astype(np.int64), np.arange(N, dtype=np.int64)])
    dst = np.concatenate([edge_index[1].astype(np.int64), np.arange(N, dtype=np.int64)])
    order = np.argsort(dst, kind="stable")
    src, dst = src[order], dst[order]
    tile_bounds = np.searchsorted(dst, np.arange(0, NTILES * P + 1, P))
    tile_cost = np.diff(tile_bounds)

    # LPT assignment of global tiles to cores (greedy, largest first)
    assign = [[] for _ in range(NC)]
    load = np.zeros(NC, dtype=np.int64)
    for g in np.argsort(-tile_cost, kind="stable"):
        c = int(np.argmin(load + (np.array([len(a) for a in assign]) >= NT) * (1 << 40)))
        assign[c].append(int(g))
        load[c] += tile_cost[g]
    core_tiles = []
    for c in range(NC):
        tl = sorted(assign[c], key=lambda g: -tile_cost[g])
        tl += [-1] * (NT - len(tl))          # dummy tiles
        core_tiles.append(tl)

    # Two node->row mappings: perm_tab gives the gather-table row (half A
    # holds tiles 0..KSPLIT-1 of every core, half B the rest, so each half
    # is the contiguous output of its own AllGather); perm_out gives the
    # local slot-major row used for x_tiles input and the output unshard.
    BROWS = (NT - KSPLIT) * P
    perm_tab = np.zeros(N, dtype=np.int64)
    perm_out = np.zeros(N, dtype=np.int64)
    for c in range(NC):
        for k, g in enumerate(core_tiles[c]):
            if g < 0:
                continue
            n0, n1 = g * P, min((g + 1) * P, N)
            if k < KSPLIT:
                base = c * KSPLIT * P + k * P
            else:
                base = HALF + c * BROWS + (k - KSPLIT) * P
            perm_tab[n0:n1] = base + np.arange(n1 - n0)
            perm_out[n0:n1] = c * NPCP + k * P + np.arange(n1 - n0)

    src_row = perm_tab[src]

    # per (core, slot): edge lists split by table half, sorted by dst
    lo_parts = [[None] * NT for _ in range(NC)]
    hi_parts = [[None] * NT for _ in range(NC)]
    for c in range(NC):
        for k, g in enumerate(core_tiles[c]):
            if g < 0:
                e0, e1 = 0, 0
                sr = np.zeros(0, dtype=np.int64)
                sl = sr
            else:
                e0, e1 = tile_bounds[g], tile_bounds[g + 1]
                sr = src_row[e0:e1]
                sl = dst[e0:e1] - g * P
            lo = sr < HALF
            lo_parts[c][k] = (sr[lo], sl[lo])
            hi_parts[c][k] = (sr[~lo] - HALF, sl[~lo])

    sched = []
    for k in range(NT):
        Lt = max(_cdiv(len(lo_parts[c][k][0]), P) for c in range(NC))
        Ht = max(_cdiv(len(hi_parts[c][k][0]), P) for c in range(NC))
        sched.append((Lt, Ht))

    TG = sum(l + h for l, h in sched)
    TI = TG * P

    idx16 = np.zeros((NC, 16, TI // 16), dtype=np.int16)
    slot_full = np.full((NC, TI), -1, dtype=np.int64)
    for c in range(NC):
        off = 0
        for k in range(NT):
            Lt, Ht = sched[k]
            for ((ss, kk), ng) in ((lo_parts[c][k], Lt), (hi_parts[c][k], Ht)):
                n = ng * P
                if n == 0:
                    continue
                si = np.zeros(n, dtype=np.int64)
                si[: len(ss)] = ss
                idx16[c, :, off // 16: (off + n) // 16] = (
                    si.reshape(-1, 16).T.astype(np.int16)
                )
                slot_full[c, off: off + len(kk)] = kk
                off += n
        assert off == TI
    idx16 = np.tile(idx16, (1, 8, 1))

    # one-hot S [e, slot] and ST [slot, e] per group, fp8
    sst = np.zeros((NC, P, TG, 2, P), dtype=ml_dtypes.float8_e4m3)
    g_ids = np.repeat(np.arange(TG), P)
    pos = np.tile(np.arange(P), TG)
    for c in range(NC):
        m = slot_full[c] >= 0
        s = slot_full[c][m]
        g = g_ids[m]
        p = pos[m]
        sst[c, p, g, 0, s] = 1.0
        sst[c, s, g, 1, p] = 1.0

    # softmax pad mask: 0 for real nodes, -1e30 for pad/dummy rows
    mask = np.full((NC, P, NT), -1e30, dtype=np.float32)
    for c in range(NC):
        for k, g in enumerate(core_tiles[c]):
            if g < 0:
                continue
            rows = min((g + 1) * P, N) - g * P
            mask[c, 0:rows, k] = 0.0

    return sched, idx16, sst, mask, core_tiles, perm_out, TI, HALF


# ----------------------------------------------------------------------------
# device kernel
# ----------------------------------------------------------------------------
def build_kernel(cfg, sched, TI, HALF, profile=False):
    import concourse.bacc as bacc
    import concourse.mybir as mybir
    import concourse.tile as tile
    from concourse.masks import make_identity

    N, IN, HID, H1, NC = cfg["N"], cfg["IN"], cfg["HID"], cfg["H1"], cfg["NCORES"]
    NPCP = NT * P
    NTAB = NC * NPCP
    OUT1 = H1 * HID                # 512
    TG = TI // P
    W2C = _cdiv(OUT1, P)           # W2 row chunks (4)
    f32, bf16 = mybir.dt.float32, mybir.dt.bfloat16
    fp8 = mybir.dt.float8e4
    i16 = mybir.dt.int16
    AX = mybir.AxisListType
    ALU = mybir.AluOpType
    ACTF = mybir.ActivationFunctionType
    RG = [list(range(NC))]

    Kt_max = max(l + h for l, h in sched)
    assert Kt_max * H1 * 4 <= 2048, "ae strip must fit one PSUM bank"

    nc = bacc.Bacc("TRN2", target_bir_lowering=False, debug=False,
                   num_devices=1 if profile else NC,
                   dynamic_dma_scratch_size=32768)

    # ---- I/O ----
    x_sl = nc.dram_tensor("x_tiles", [NPCP, IN], f32, kind="ExternalInput")
    W1_d = nc.dram_tensor("W1", [IN, OUT1], f32, kind="ExternalInput")
    as1_d = nc.dram_tensor("att_src1", [H1, HID], f32, kind="ExternalInput")
    ad1_d = nc.dram_tensor("att_dst1", [H1, HID], f32, kind="ExternalInput")
    b1_d = nc.dram_tensor("b1", [OUT1], f32, kind="ExternalInput")
    W2_d = nc.dram_tensor("W2", [OUT1, HID], f32, kind="ExternalInput")
    as2_d = nc.dram_tensor("att_src2", [1, HID], f32, kind="ExternalInput")
    ad2_d = nc.dram_tensor("att_dst2", [1, HID], f32, kind="ExternalInput")
    b2_d = nc.dram_tensor("b2", [HID], f32, kind="ExternalInput")
    fcw_d = nc.dram_tensor("fc_w", [HID, 1], f32, kind="ExternalInput")
    fcb_d = nc.dram_tensor("fc_b", [1], f32, kind="ExternalInput")
    idx_d = nc.dram_tensor("idx16", [P, TI // 16], i16, kind="ExternalInput")
    sst_d = nc.dram_tensor("sst", [P, TG, 2, P], fp8, kind="ExternalInput")
    mask_d = nc.dram_tensor("padmask", [P, NT], f32, kind="ExternalInput")
    out_d = nc.dram_tensor("out", [NPCP, 1], f32, kind="ExternalOutput")

    # ---- internal DRAM ----
    ASL = KSPLIT * P
    h1tab_inA = nc.dram_tensor("h1tab_inA", [ASL, TW1], bf16)
    h1tab_inB = nc.dram_tensor("h1tab_inB", [NPCP - ASL, TW1], bf16)
    h1tab = nc.dram_tensor("h1tab", [NTAB, TW1], bf16, addr_space="Shared")
    h2tab_inA = nc.dram_tensor("h2tab_inA", [ASL, TW2], bf16)
    h2tab_inB = nc.dram_tensor("h2tab_inB", [NPCP - ASL, TW2], bf16)
    h2tab = nc.dram_tensor("h2tab", [NTAB, TW2], bf16, addr_space="Shared")
    ssum_in = nc.dram_tensor("ssum_in", [1, 1], f32)
    ssum = nc.dram_tensor("ssum", [1, 1], f32, addr_space="Shared")

    with tile.TileContext(nc) as tc, ExitStack() as ctx:
        const = ctx.enter_context(tc.tile_pool(name="const", bufs=1))
        sb = ctx.enter_context(tc.tile_pool(name="sb", bufs=2))
        sb3 = ctx.enter_context(tc.tile_pool(name="sb3", bufs=3))
        psA = ctx.enter_context(tc.tile_pool(name="psA", bufs=2, space="PSUM"))
        psB = ctx.enter_context(tc.tile_pool(name="psB", bufs=2, space="PSUM"))

        # ================= constants / weights =================
        idbf = const.tile([P, P], bf16)
        make_identity(nc, idbf[:])
        ones_r = const.tile([1, P], f32)
        nc.vector.memset(ones_r[:], 1.0)
        ones_c = const.tile([P, 1], f32)
        nc.vector.memset(ones_c[:], 1.0)

        def bcast_row(dram_ap, width, name):
            row = sb.tile([1, width], f32, tag="bcrow")
            nc.sync.dma_start(row[:], dram_ap)
            pt = psB.tile([P, width], f32, tag="tp")
            nc.tensor.matmul(pt[:], lhsT=ones_r[:], rhs=row[:], start=True,
                             stop=True)
            out = const.tile([P, width], f32, tag=name)
            nc.scalar.copy(out[:], pt[:])
            return out

        # Only the constants phase 0 actually consumes are staged before the
        # table-build loop; everything else is emitted after it so those
        # loads/broadcasts overlap the loop's pipeline.
        w1f = const.tile([P, OUT1], f32)
        nc.sync.dma_start(w1f[:], W1_d.ap())
        w1b = const.tile([P, OUT1], bf16)
        nc.vector.tensor_copy(w1b[:], w1f[:])
        att1s_bc = bcast_row(
            as1_d.ap().rearrange("(o h) d -> o (h d)", o=1), OUT1, "a1s")
        att1d_bc = bcast_row(
            ad1_d.ap().rearrange("(o h) d -> o (h d)", o=1), OUT1, "a1d")

        # wfold[:, 0:8] = per-head fold of att_src1 into W1 columns;
        # wfold[:, 8:16] same for att_dst1:  a_src1(n) = x(n) @ wfold[:,0:8]
        wfold = const.tile([P, 2 * H1], bf16)
        for ci, att_bc in ((0, att1s_bc), (1, att1d_bc)):
            tmp = sb.tile([P, OUT1], f32, tag="tmpw2")
            nc.vector.tensor_tensor(tmp[:], w1f[:], att_bc[:], op=ALU.mult)
            red = sb.tile([P, H1], f32, tag="tmpw3")
            nc.vector.tensor_reduce(
                red[:], tmp[:].rearrange("p (h d) -> p h d", h=H1),
                axis=AX.X, op=ALU.add)
            nc.vector.tensor_copy(wfold[:, ci * H1:(ci + 1) * H1], red[:])

        adst_sb = const.tile([P, NT, H1], bf16)    # a_dst1 per local node
        adn2_sb = const.tile([P, NT, 1], bf16)     # a_dst2 per local node
        logits = const.tile([P, NT], f32, tag="logits")

        # ============ phase 0: build layer-1 table rows for local slice ====
        XB = 4   # x tiles per load
        for k in range(NT):
            r0 = k * P
            if k % XB == 0:
                kn = min(XB, NT - k)
                xf = sb3.tile([P, XB, IN], f32, tag="xf")
                nc.sync.dma_start(
                    xf[:, 0:kn, :],
                    x_sl.ap()[r0:r0 + kn * P, :].rearrange(
                        "(t p) f -> p t f", p=P))
            xb = sb3.tile([P, IN], bf16, tag="xb")
            nc.vector.tensor_copy(xb[:], xf[:, k % XB, :])
            xt_p = psB.tile([P, P], bf16, tag="tp")
            nc.tensor.transpose(xt_p[:], xb[:], idbf[:])
            xt = sb3.tile([P, P], bf16, tag="xt")
            nc.scalar.copy(xt[:], xt_p[:])
            h1_p = psA.tile([P, OUT1], f32, tag="zbig")
            nc.tensor.matmul(h1_p[:], lhsT=xt[:], rhs=w1b[:], start=True,
                             stop=True)
            fold_p = psA.tile([P, 2 * H1], f32, tag="ae")
            nc.tensor.matmul(fold_p[:], lhsT=xt[:], rhs=wfold[:], start=True,
                             stop=True)
            trow = sb.tile([P, TW1], bf16, tag="trow", bufs=3)
            nc.vector.tensor_copy(trow[:, 0:OUT1], h1_p[:])
            nc.scalar.copy(trow[:, OUT1:OUT1 + H1], fold_p[:, 0:H1])
            nc.vector.tensor_copy(adst_sb[:, k, :], fold_p[:, H1:2 * H1])
            if k < KSPLIT:
                nc.scalar.dma_start(h1tab_inA.ap()[r0:r0 + P, :], trow[:])
            else:
                nc.scalar.dma_start(
                    h1tab_inB.ap()[r0 - ASL:r0 - ASL + P, :], trow[:])

        # late constants: loaded/broadcast while the table build drains
        idx16_sb = const.tile([P, TI // 16], i16)
        nc.sync.dma_start(idx16_sb[:], idx_d.ap())
        mask_sb = const.tile([P, NT], f32)
        nc.sync.dma_start(mask_sb[:], mask_d.ap())
        w2b = const.tile([P, W2C, HID], bf16)
        w2f_t = sb.tile([P, W2C, HID], f32, tag="tmpw")
        nc.sync.dma_start(
            w2f_t[:], W2_d.ap().rearrange("(c p) n -> p c n", p=P))
        nc.vector.tensor_copy(w2b[:], w2f_t[:])
        att2s_bc = bcast_row(as2_d.ap(), HID, "a2s")
        att2d_bc = bcast_row(ad2_d.ap(), HID, "a2d")
        b1_bc = bcast_row(b1_d.ap()[None, :], OUT1, "b1")
        b2_bc = bcast_row(b2_d.ap()[None, :], HID, "b2")
        fcb_bc = bcast_row(fcb_d.ap()[None, :], 1, "fcb")
        fcw_f = sb.tile([HID, 1], f32, tag="tmpw4")
        nc.sync.dma_start(fcw_f[:], fcw_d.ap())
        fcw_sb = const.tile([HID, 1], bf16)
        nc.vector.tensor_copy(fcw_sb[:], fcw_f[:])

        if profile:
            nc.sync.dma_start(h1tab.ap()[0:ASL, :], h1tab_inA.ap())
            nc.sync.dma_start(h1tab.ap()[HALF:HALF + NPCP - ASL, :],
                              h1tab_inB.ap())
        else:
            nc.gpsimd.collective_compute(
                "AllGather", ALU.bypass, replica_groups=RG,
                ins=[h1tab_inA.ap().opt()],
                outs=[h1tab.ap()[0:HALF, :].opt()])
            nc.gpsimd.collective_compute(
                "AllGather", ALU.bypass, replica_groups=RG,
                ins=[h1tab_inB.ap().opt()],
                outs=[h1tab.ap()[HALF:NTAB, :].opt()])

        # ================= shared edge-phase machinery =====================
        def edge_phase(layer):
            L1 = layer == 1
            table = h1tab if L1 else h2tab
            FW = TW1 if L1 else TW2    # table row cols
            NH = H1 if L1 else 1       # heads
            ZC = OUT1 if L1 else HID + 1   # z matmul cols
            lo_ap = table.ap()[0:HALF, :]
            hi_ap = table.ap()[HALF:NTAB, :]
            sfx = "1" if L1 else "2"
            GCHL = GCH if L1 else GCH2
            goff = 0
            for t in range(NT):
                Lt, Ht = sched[t]
                Kt = Lt + Ht
                gbase = t * P

                # --- one-hot S/ST ship + gathers ---
                sst_sb = sb.tile([P, Kt, 2, P], fp8, tag="sst" + sfx)
                sst_eng = nc.scalar if L1 else nc.sync
                sst_eng.dma_start(sst_sb[:], sst_d.ap()[:, goff:goff + Kt])
                X_all = sb.tile([P, Kt, FW], bf16, tag="X" + sfx)
                off16 = goff * P // 16
                for ci, (g0, gn, half_ap) in enumerate(
                        [(q, min(GCHL, Lt - q), lo_ap)
                         for q in range(0, Lt, GCHL)]
                        + [(Lt + q, min(GCHL, Ht - q), hi_ap)
                           for q in range(0, Ht, GCHL)]):
                    n = gn * P
                    idxs = idx16_sb[:, off16 + g0 * P // 16:
                                    off16 + (g0 * P + n) // 16]
                    nc.gpsimd.dma_gather(
                        X_all[:, g0:g0 + gn, :], half_ap, idxs, n, n, FW,
                        transpose=False)

                # --- per-edge logits for the whole tile (in one PSUM strip):
                # a_dst via one small matmul per group, then the gathered
                # a_src accumulated on top via an identity-lhsT matmul.
                ae_p = psA.tile([P, Kt, NH], f32, tag="ae")
                adn = (adst_sb if L1 else adn2_sb)[:, t, :]
                for j in range(Kt):
                    nc.tensor.matmul(ae_p[:, j, :], lhsT=sst_sb[:, j, 1, :],
                                     rhs=adn, start=(j == 0), stop=False)
                asrc = (X_all[:, :, OUT1:OUT1 + H1] if L1
                        else X_all[:, :, HID + 1:HID + 2])
                nc.tensor.matmul(ae_p[:], lhsT=idbf[:], rhs=asrc,
                                 start=False, stop=True)
                ae_sb = sb.tile([P, Kt, NH], f32, tag="aesb" + sfx, bufs=3)
                nc.scalar.copy(ae_sb[:].rearrange("p k h -> p (k h)"),
                               ae_p[:].rearrange("p k h -> p (k h)"))
                lr = sb.tile([P, Kt, NH], f32, tag="lr" + sfx, bufs=3)
                nc.vector.scalar_tensor_tensor(
                    lr[:], in0=ae_sb[:], scalar=0.2, in1=ae_sb[:],
                    op0=ALU.mult, op1=ALU.max)
                p_all = sb.tile([P, Kt, NH], bf16, tag="p" + sfx, bufs=4)
                nc.scalar.activation(
                    p_all[:].rearrange("p k h -> p (k h)"),
                    lr[:].rearrange("p k h -> p (k h)"), ACTF.Exp)

                # --- segment sums via one-hot matmuls ---
                z_p = psA.tile([P, ZC], f32, tag="zbig")
                if L1:
                    s_p = psA.tile([P, H1], f32, tag="ae")
                    pdup = sb.tile([P, Kt, H1, 2], bf16, tag="pdup", bufs=3)
                    nc.vector.tensor_copy(
                        pdup[:],
                        p_all[:, :, :, None].to_broadcast([P, Kt, H1, 2]))
                for m0 in range(0, Kt, MCH):
                    mk = min(MCH, Kt - m0)
                    if L1:
                        M_c = sb.tile([P, MCH, H1, 32, 2], bf16, tag="M", bufs=3)
                        nc.vector.tensor_tensor(
                            M_c[:, 0:mk],
                            X_all[:, m0:m0 + mk, 0:OUT1].rearrange(
                                "p k (h f g) -> p k h f g", h=H1, g=2),
                            pdup[:, m0:m0 + mk, :, None, :].to_broadcast(
                                [P, mk, H1, 32, 2]),
                            op=ALU.mult)
                    else:
                        M_c = sb.tile([P, MCH, ZC], bf16, tag="M2", bufs=3)
                        nc.vector.tensor_tensor(
                            M_c[:, 0:mk],
                            X_all[:, m0:m0 + mk, 0:ZC],
                            p_all[:, m0:m0 + mk, :].to_broadcast([P, mk, ZC]),
                            op=ALU.mult)
                    for j in range(m0, m0 + mk):
                        rhs = (M_c[:, j - m0].rearrange("p h f g -> p (h f g)")
                               if L1 else M_c[:, j - m0])
                        nc.tensor.matmul(z_p[:], lhsT=sst_sb[:, j, 0, :],
                                         rhs=rhs, start=(j == 0),
                                         stop=(j == Kt - 1))
                        if L1:
                            nc.tensor.matmul(s_p[:], lhsT=sst_sb[:, j, 0, :],
                                             rhs=p_all[:, j, :],
                                             start=(j == 0),
                                             stop=(j == Kt - 1))
                goff += Kt

                # ---------------- finalize dst tile ----------------
                if L1:
                    s_eps = sb.tile([P, H1], f32, tag="seps")
                    nc.vector.tensor_scalar(s_eps[:], s_p[:], 1e-16, None,
                                            op0=ALU.add)
                    s_inv = sb.tile([P, H1], f32, tag="sinv", bufs=3)
                    nc.vector.reciprocal(s_inv[:], s_eps[:])
                    y = sb.tile([P, OUT1], f32, tag="y")
                    nc.vector.tensor_tensor(
                        y[:].rearrange("p (h f) -> p h f", h=H1),
                        z_p[:].rearrange("p (h f) -> p h f", h=H1),
                        s_inv[:, :, None].to_broadcast([P, H1, HID]),
                        op=ALU.mult)
                    WE = OUT1
                    yb = sb.tile([P, WE], f32, tag="yb")
                    nc.vector.tensor_tensor(yb[:], y[:], b1_bc[:], op=ALU.add)
                else:
                    s2e = sb.tile([P, 1], f32, tag="s2e")
                    nc.vector.tensor_scalar(s2e[:], z_p[:, HID:HID + 1],
                                            1e-16, None, op0=ALU.add)
                    s2i = sb.tile([P, 1], f32, tag="s2i")
                    nc.vector.reciprocal(s2i[:], s2e[:])
                    y = sb.tile([P, HID], f32, tag="y2")
                    nc.vector.tensor_scalar(y[:], z_p[:, 0:HID], s2i[:, 0:1],
                                            None, op0=ALU.mult)
                    WE = HID
                    yb = sb.tile([P, WE], f32, tag="yb2")
                    nc.vector.tensor_tensor(yb[:], y[:], b2_bc[:], op=ALU.add)
                # elu(yb) = relu(yb) + exp(min(yb,0)) - 1
                t0 = sb.tile([P, WE], f32, tag="elu0" + sfx)
                nc.vector.tensor_scalar_min(t0[:], yb[:], 0.0)
                ex = sb.tile([P, WE], f32, tag="elu1" + sfx)
                nc.scalar.activation(ex[:], t0[:], ACTF.Exp)
                ry = sb.tile([P, WE], f32, tag="elu2" + sfx)
                nc.scalar.activation(ry[:], yb[:], ACTF.Relu)
                x2 = sb.tile([P, WE], bf16, tag="x2" + sfx, bufs=3)
                nc.vector.scalar_tensor_tensor(
                    x2[:], in0=ex[:], scalar=-1.0, in1=ry[:],
                    op0=ALU.add, op1=ALU.add)

                if L1:
                    # h2 = x2 @ W2, then pack layer-2 table row
                    h2_p = psA.tile([P, HID], f32, tag="zbig")
                    for cix in range(W2C):
                        xt_p = psB.tile([P, P], bf16, tag="tp")
                        nc.tensor.transpose(
                            xt_p[:], x2[:, cix * P:(cix + 1) * P], idbf[:])
                        xt2 = sb3.tile([P, P], bf16, tag="xt2")
                        nc.scalar.copy(xt2[:], xt_p[:])
                        nc.tensor.matmul(h2_p[:], lhsT=xt2[:],
                                         rhs=w2b[:, cix, :],
                                         start=(cix == 0),
                                         stop=(cix == W2C - 1))
                    trow2 = sb.tile([P, TW2], bf16, tag="trow2", bufs=3)
                    nc.scalar.copy(trow2[:, 0:HID], h2_p[:])
                    nc.vector.memset(trow2[:, HID:HID + 1], 1.0)
                    tmp = sb.tile([P, HID], f32, tag="atmp")
                    ared = sb.tile([P, 1], f32, tag="ared")
                    nc.vector.tensor_tensor(tmp[:], h2_p[:], att2s_bc[:],
                                            op=ALU.mult)
                    nc.vector.tensor_reduce(ared[:], tmp[:], axis=AX.X,
                                            op=ALU.add)
                    nc.vector.tensor_copy(trow2[:, HID + 1:HID + 2], ared[:])
                    ared2 = sb.tile([P, 1], f32, tag="ared2")
                    nc.vector.tensor_tensor(tmp[:], h2_p[:], att2d_bc[:],
                                            op=ALU.mult)
                    nc.vector.tensor_reduce(ared2[:], tmp[:], axis=AX.X,
                                            op=ALU.add)
                    nc.vector.tensor_copy(adn2_sb[:, t, :], ared2[:])
                    if t < KSPLIT:
                        nc.scalar.dma_start(
                            h2tab_inA.ap()[gbase:gbase + P, :], trow2[:])
                    else:
                        nc.scalar.dma_start(
                            h2tab_inB.ap()[gbase - ASL:gbase - ASL + P, :],
                            trow2[:])
                else:
                    x2t_p = psB.tile([HID, P], bf16, tag="tp")
                    nc.tensor.transpose(x2t_p[:], x2[:, 0:HID], idbf[:])
                    x2t = sb3.tile([HID, P], bf16, tag="x2t")
                    nc.scalar.copy(x2t[:], x2t_p[:])
                    lg_p = psB.tile([P, 1], f32, tag="tp")
                    nc.tensor.matmul(lg_p[:], lhsT=x2t[:], rhs=fcw_sb[:],
                                     start=True, stop=True)
                    nc.scalar.activation(logits[:, t:t + 1],
                                         lg_p[:], ACTF.Identity,
                                         bias=fcb_bc[:])

        # ================= layer 1 =================
        edge_phase(1)
        if profile:
            nc.sync.dma_start(h2tab.ap()[0:ASL, :], h2tab_inA.ap())
            nc.sync.dma_start(h2tab.ap()[HALF:HALF + NPCP - ASL, :],
                              h2tab_inB.ap())
        else:
            nc.gpsimd.collective_compute(
                "AllGather", ALU.bypass, replica_groups=RG,
                ins=[h2tab_inA.ap().opt()],
                outs=[h2tab.ap()[0:HALF, :].opt()])
            nc.gpsimd.collective_compute(
                "AllGather", ALU.bypass, replica_groups=RG,
                ins=[h2tab_inB.ap().opt()],
                outs=[h2tab.ap()[HALF:NTAB, :].opt()])

        # ================= layer 2 =================
        edge_phase(2)

        # ================= softmax over all nodes =================
        lgm = sb.tile([P, NT], f32, tag="lgm")
        nc.vector.tensor_tensor(lgm[:], logits[:], mask_sb[:], op=ALU.add)
        ex_all = sb.tile([P, NT], f32, tag="exall")
        nc.scalar.activation(ex_all[:], lgm[:], ACTF.Exp)
        part = sb.tile([P, 1], f32, tag="part")
        nc.vector.tensor_reduce(part[:], ex_all[:], axis=AX.X, op=ALU.add)
        tot_p = psB.tile([1, 1], f32, tag="tp")
        nc.tensor.matmul(tot_p[:], lhsT=part[:], rhs=ones_c[:], start=True,
                         stop=True)
        tot_sb = sb.tile([1, 1], f32, tag="tot")
        nc.scalar.copy(tot_sb[:], tot_p[:])
        nc.sync.dma_start(ssum_in.ap(), tot_sb[:])
        if profile:
            nc.sync.dma_start(ssum.ap(), ssum_in.ap())
        else:
            nc.gpsimd.collective_compute(
                "AllReduce", ALU.add, replica_groups=RG,
                ins=[ssum_in.ap().opt()], outs=[ssum.ap().opt()])
        gsum = sb.tile([1, 1], f32, tag="gsum")
        nc.sync.dma_start(gsum[:], ssum.ap())
        ginv = sb.tile([1, 1], f32, tag="ginv")
        nc.vector.reciprocal(ginv[:], gsum[:])
        ginv_p = psB.tile([P, 1], f32, tag="tp")
        nc.tensor.matmul(ginv_p[:], lhsT=ones_r[:], rhs=ginv[:], start=True,
                         stop=True)
        ginv_bc = sb.tile([P, 1], f32, tag="ginvbc")
        nc.scalar.copy(ginv_bc[:], ginv_p[:])
        res = sb.tile([P, NT], f32, tag="res")
        nc.vector.tensor_scalar(res[:], ex_all[:], ginv_bc[:], None,
                                op0=ALU.mult)
        nc.sync.dma_start(
            out_d.ap().rearrange("(t p) o -> p (t o)", p=P), res[:])

    nc.compile()
    return nc


# ----------------------------------------------------------------------------
# entry point
# ----------------------------------------------------------------------------
def build_in_maps(inputs, cfg):
    sched, idx16, sst, mask, core_tiles, perm, TI, HALF = prep_edges(
        np.asarray(inputs["edge_index"]), cfg)
    x = np.asarray(inputs["x"], dtype=np.float32)
    N, NC = cfg["N"], cfg["NCORES"]
    NPCP = NT * P
    common = {k: np.ascontiguousarray(np.asarray(inputs[k], np.float32))
              for k in ("W1", "att_src1", "att_dst1", "b1", "W2", "att_src2",
                        "att_dst2", "b2", "fc_w", "fc_b")}
    xperm = np.zeros((NC * NPCP, x.shape[1]), dtype=np.float32)
    xperm[perm] = x
    in_maps = []
    for c in range(NC):
        m = dict(common)
        m["x_tiles"] = np.ascontiguousarray(xperm[c * NPCP:(c + 1) * NPCP])
        m["idx16"] = np.ascontiguousarray(idx16[c])
        m["sst"] = np.ascontiguousarray(sst[c])
        m["padmask"] = np.ascontiguousarray(mask[c])
        in_maps.append(m)
    return in_maps, sched, TI, HALF, perm


def kernel(**inputs) -> np.ndarray:
    from concourse import bass_utils

    cfg = dict(CFG)
    in_maps, sched, TI, HALF, perm = build_in_maps(inputs, cfg)
    nc = build_kernel(cfg, sched, TI, HALF)
    res = bass_utils.run_bass_kernel_spmd(
        nc, in_maps, core_ids=list(range(cfg["NCORES"])),
        trace=bool(int(os.environ.get("GAT_TRACE", "0"))))
    kernel.last_results = res
    allout = np.concatenate([r["out"] for r in res.results], axis=0)
    return allout[perm].astype(np.float32)


# revision 51
# speedup vs baseline: 1.1835x; 1.1835x over previous
"""Two-layer GAT (PyG-style, eval mode) on 8 Trainium2 NeuronCores.

Sharding: dst nodes are processed in 128-node tiles; tiles are LPT-assigned
to cores to balance padded edge-group counts (the SPMD schedule is the
per-slot max over cores, so each core also sorts its tiles by size).  The
node->table-row permutation this induces is applied host-side to the gather
indices and inverted on the output.

Design (v2, instruction-count-driven):
- Layer-1 aggregates in h1-space: a per-node table row holds
  [h1 (8 heads x 64) | a_src (8) | pad] bf16 (640 cols = 1280 B, the 256 B
  gather granularity), built per local slice then AllGathered in two halves
  (each half overlaps other work).  One dma_gather per 4 (layer 1) / 8
  (layer 2) edge groups; a_src rides the table row, so the baseline's
  transposed per-edge gather is gone.
- One-hot selection matrices S ([edge, slot]) and their transposes ST
  ([slot, edge]) are host-precomputed in fp8e4 and DMA-shipped per tile:
  no per-group is_equal builds and no per-group PE transposes.  ST slices
  feed per-group a_dst matmuls (lhsT=ST_j, rhs=a_dst_node) into one PSUM
  strip for the whole tile, and an identity-lhsT matmul accumulates the
  gathered a_src on top; S slices are the lhsT of the segment-sum (z) and
  softmax-denominator (s) matmuls.
- Per-edge softmax weights for a whole tile: one DVE leaky-relu (reading
  the logits straight from PSUM) and one ScalarE exp.
- The message product M = h1_gathered * p uses a duplicated-pair access
  pattern (innermost [2]-stride-1 on every operand) to hit the DVE 2x
  16-bit mode.
- Layer-2 table rows: [h2 (64) | 1.0 | a_src2 | pad] (256 B).  The ones
  column makes the z matmul produce the softmax denominator for free.
segment-max is skipped: logits are O(+-8) so exp() is safe in fp32, and
softmax is shift-invariant so the result matches the reference.
"""

import os
from contextlib import ExitStack

import numpy as np

# ----------------------------------------------------------------------------
# problem config (hardcoded per contest contract)
# ----------------------------------------------------------------------------
CFG = dict(
    N=50000,       # nodes
    IN=128,        # input feature dim
    HID=64,        # per-head hidden dim
    H1=8,          # layer-1 heads
    NCORES=8,
)

P = 128    # partitions / tile edge
GCH = 4    # edge groups per dma_gather chunk, layer 1
GCH2 = 8   # edge groups per dma_gather chunk, layer 2 (1024 rows)
MCH = 8    # edge groups per message-product chunk
TW1 = 640  # layer-1 table row cols (bf16): 512 h1 + 8 a_src + pad
TW2 = 128  # layer-2 table row cols: 64 h2 + 1 ones + 1 a_src2 + pad
NT = 49    # dst tiles per core (8*49*128 = 50176 >= 50000)
KSPLIT = 27  # tiles per core in table half A (AllGather split point)


def _cdiv(a, b):
    return (a + b - 1) // b


# ----------------------------------------------------------------------------
# host-side sharding prep (pure layout work: sort, bucket, pad, pack indices)
# ----------------------------------------------------------------------------
def prep_edges(edge_index, cfg):
    """Sort self-loop-augmented edges by dst; LPT-assign 128-dst-node tiles
    to cores balancing padded group counts; build per-core packed gather
    indices (lo/hi table halves for the int16 limit), fp8 one-hot S/ST
    selection matrices, the node->table-row permutation, and the softmax
    pad-row masks.
    """
    import ml_dtypes

    N, NC = cfg["N"], cfg["NCORES"]
    NPCP = NT * P                  # padded rows per core (6272)
    NTAB = NC * NPCP               # permuted table rows (50176)
    HALF = NC * KSPLIT * P         # rows in table half A (= lo/hi boundary)
    NTILES = _cdiv(N, P)           # 391 global tiles

    src = np.concatenate([edge_index[0].astype(np.int64), np.arange(N, dtype=np.int64)])
    dst = np.concatenate([edge_index[1].astype(np.int64), np.arange(N, dtype=np.int64)])
    order = np.argsort(dst, kind="stable")
    src, dst = src[order], dst[order]
    tile_bounds = np.searchsorted(dst, np.arange(0, NTILES * P + 1, P))
    tile_cost = np.diff(tile_bounds)

    # LPT assignment of global tiles to cores (greedy, largest first)
    assign = [[] for _ in range(NC)]
    load = np.zeros(NC, dtype=np.int64)
    for g in np.argsort(-tile_cost, kind="stable"):
        c = int(np.argmin(load + (np.array([len(a) for a in assign]) >= NT) * (1 << 40)))
        assign[c].append(int(g))
        load[c] += tile_cost[g]
    core_tiles = []
    for c in range(NC):
        tl = sorted(assign[c], key=lambda g: -tile_cost[g])
        tl += [-1] * (NT - len(tl))          # dummy tiles
        core_tiles.append(tl)

    # Two node->row mappings: perm_tab gives the gather-table row (half A
    # holds tiles 0..KSPLIT-1 of every core, half B the rest, so each half
    # is the contiguous output of its own AllGather); perm_out gives the
    # local slot-major row used for x_tiles input and the output unshard.
    BROWS = (NT - KSPLIT) * P
    perm_tab = np.zeros(N, dtype=np.int64)
    perm_out = np.zeros(N, dtype=np.int64)
    for c in range(NC):
        for k, g in enumerate(core_tiles[c]):
            if g < 0:
                continue
            n0, n1 = g * P, min((g + 1) * P, N)
            if k < KSPLIT:
                base = c * KSPLIT * P + k * P
            else:
                base = HALF + c * BROWS + (k - KSPLIT) * P
            perm_tab[n0:n1] = base + np.arange(n1 - n0)
            perm_out[n0:n1] = c * NPCP + k * P + np.arange(n1 - n0)

    src_row = perm_tab[src]

    # per (core, slot): edge lists split by table half, sorted by dst
    lo_parts = [[None] * NT for _ in range(NC)]
    hi_parts = [[None] * NT for _ in range(NC)]
    for c in range(NC):
        for k, g in enumerate(core_tiles[c]):
            if g < 0:
                e0, e1 = 0, 0
                sr = np.zeros(0, dtype=np.int64)
                sl = sr
            else:
                e0, e1 = tile_bounds[g], tile_bounds[g + 1]
                sr = src_row[e0:e1]
                sl = dst[e0:e1] - g * P
            lo = sr < HALF
            lo_parts[c][k] = (sr[lo], sl[lo])
            hi_parts[c][k] = (sr[~lo] - HALF, sl[~lo])

    sched = []
    for k in range(NT):
        Lt = max(_cdiv(len(lo_parts[c][k][0]), P) for c in range(NC))
        Ht = max(_cdiv(len(hi_parts[c][k][0]), P) for c in range(NC))
        sched.append((Lt, Ht))

    TG = sum(l + h for l, h in sched)
    TI = TG * P

    idx16 = np.zeros((NC, 16, TI // 16), dtype=np.int16)
    slot_full = np.full((NC, TI), -1, dtype=np.int64)
    for c in range(NC):
        off = 0
        for k in range(NT):
            Lt, Ht = sched[k]
            for ((ss, kk), ng) in ((lo_parts[c][k], Lt), (hi_parts[c][k], Ht)):
                n = ng * P
                if n == 0:
                    continue
                si = np.zeros(n, dtype=np.int64)
                si[: len(ss)] = ss
                idx16[c, :, off // 16: (off + n) // 16] = (
                    si.reshape(-1, 16).T.astype(np.int16)
                )
                slot_full[c, off: off + len(kk)] = kk
                off += n
        assert off == TI
    idx16 = np.tile(idx16, (1, 8, 1))

    # one-hot S [e, slot] and ST [slot, e] per group, fp8
    sst = np.zeros((NC, P, TG, 2, P), dtype=ml_dtypes.float8_e4m3)
    g_ids = np.repeat(np.arange(TG), P)
    pos = np.tile(np.arange(P), TG)
    for c in range(NC):
        m = slot_full[c] >= 0
        s = slot_full[c][m]
        g = g_ids[m]
        p = pos[m]
        sst[c, p, g, 0, s] = 1.0
        sst[c, s, g, 1, p] = 1.0

    # softmax pad mask: 0 for real nodes, -1e30 for pad/dummy rows
    mask = np.full((NC, P, NT), -1e30, dtype=np.float32)
    for c in range(NC):
        for k, g in enumerate(core_tiles[c]):
            if g < 0:
                continue
            rows = min((g + 1) * P, N) - g * P
            mask[c, 0:rows, k] = 0.0

    return sched, idx16, sst, mask, core_tiles, perm_out, TI, HALF


# ----------------------------------------------------------------------------
# device kernel
# ----------------------------------------------------------------------------
def build_kernel(cfg, sched, TI, HALF, profile=False):
    import concourse.bacc as bacc
    import concourse.mybir as mybir
    import concourse.tile as tile
    from concourse.masks import make_identity

    N, IN, HID, H1, NC = cfg["N"], cfg["IN"], cfg["HID"], cfg["H1"], cfg["NCORES"]
    NPCP = NT * P
    NTAB = NC * NPCP
    OUT1 = H1 * HID                # 512
    TG = TI // P
    W2C = _cdiv(OUT1, P)           # W2 row chunks (4)
    f32, bf16 = mybir.dt.float32, mybir.dt.bfloat16
    fp8 = mybir.dt.float8e4
    i16 = mybir.dt.int16
    AX = mybir.AxisListType
    ALU = mybir.AluOpType
    ACTF = mybir.ActivationFunctionType
    RG = [list(range(NC))]

    Kt_max = max(l + h for l, h in sched)
    assert Kt_max * H1 * 4 <= 2048, "ae strip must fit one PSUM bank"
    # deep pipelining fits SBUF only for balanced schedules; degrade
    # gracefully on inputs with fatter tiles
    B3 = 3 if Kt_max <= 19 else 2
    B4 = 4 if Kt_max <= 19 else 2

    nc = bacc.Bacc("TRN2", target_bir_lowering=False, debug=False,
                   num_devices=1 if profile else NC,
                   dynamic_dma_scratch_size=32768 if B3 == 3 else 16384)

    # ---- I/O ----
    x_sl = nc.dram_tensor("x_tiles", [NPCP, IN], f32, kind="ExternalInput")
    W1_d = nc.dram_tensor("W1", [IN, OUT1], f32, kind="ExternalInput")
    as1_d = nc.dram_tensor("att_src1", [H1, HID], f32, kind="ExternalInput")
    ad1_d = nc.dram_tensor("att_dst1", [H1, HID], f32, kind="ExternalInput")
    b1_d = nc.dram_tensor("b1", [OUT1], f32, kind="ExternalInput")
    W2_d = nc.dram_tensor("W2", [OUT1, HID], f32, kind="ExternalInput")
    as2_d = nc.dram_tensor("att_src2", [1, HID], f32, kind="ExternalInput")
    ad2_d = nc.dram_tensor("att_dst2", [1, HID], f32, kind="ExternalInput")
    b2_d = nc.dram_tensor("b2", [HID], f32, kind="ExternalInput")
    fcw_d = nc.dram_tensor("fc_w", [HID, 1], f32, kind="ExternalInput")
    fcb_d = nc.dram_tensor("fc_b", [1], f32, kind="ExternalInput")
    idx_d = nc.dram_tensor("idx16", [P, TI // 16], i16, kind="ExternalInput")
    sst_d = nc.dram_tensor("sst", [P, TG, 2, P], fp8, kind="ExternalInput")
    mask_d = nc.dram_tensor("padmask", [P, NT], f32, kind="ExternalInput")
    out_d = nc.dram_tensor("out", [NPCP, 1], f32, kind="ExternalOutput")

    # ---- internal DRAM ----
    ASL = KSPLIT * P
    h1tab_inA = nc.dram_tensor("h1tab_inA", [ASL, TW1], bf16)
    h1tab_inB = nc.dram_tensor("h1tab_inB", [NPCP - ASL, TW1], bf16)
    h1tab = nc.dram_tensor("h1tab", [NTAB, TW1], bf16, addr_space="Shared")
    h2tab_inA = nc.dram_tensor("h2tab_inA", [ASL, TW2], bf16)
    h2tab_inB = nc.dram_tensor("h2tab_inB", [NPCP - ASL, TW2], bf16)
    h2tab = nc.dram_tensor("h2tab", [NTAB, TW2], bf16, addr_space="Shared")
    ssum_in = nc.dram_tensor("ssum_in", [1, 1], f32)
    ssum = nc.dram_tensor("ssum", [1, 1], f32, addr_space="Shared")

    with tile.TileContext(nc) as tc, ExitStack() as ctx:
        const = ctx.enter_context(tc.tile_pool(name="const", bufs=1))
        sb = ctx.enter_context(tc.tile_pool(name="sb", bufs=2))
        sb3 = ctx.enter_context(tc.tile_pool(name="sb3", bufs=3))
        psA = ctx.enter_context(tc.tile_pool(name="psA", bufs=2, space="PSUM"))
        psB = ctx.enter_context(tc.tile_pool(name="psB", bufs=2, space="PSUM"))

        # ================= constants / weights =================
        idbf = const.tile([P, P], bf16)
        make_identity(nc, idbf[:])
        ones_r = const.tile([1, P], f32)
        nc.vector.memset(ones_r[:], 1.0)
        ones_c = const.tile([P, 1], f32)
        nc.vector.memset(ones_c[:], 1.0)

        def bcast_row(dram_ap, width, name):
            row = sb.tile([1, width], f32, tag="bcrow")
            nc.sync.dma_start(row[:], dram_ap)
            pt = psB.tile([P, width], f32, tag="tp")
            nc.tensor.matmul(pt[:], lhsT=ones_r[:], rhs=row[:], start=True,
                             stop=True)
            out = const.tile([P, width], f32, tag=name)
            nc.scalar.copy(out[:], pt[:])
            return out

        # Only the constants phase 0 actually consumes are staged before the
        # table-build loop; everything else is emitted after it so those
        # loads/broadcasts overlap the loop's pipeline.
        w1f = const.tile([P, OUT1], f32)
        nc.sync.dma_start(w1f[:], W1_d.ap())
        w1b = const.tile([P, OUT1], bf16)
        nc.vector.tensor_copy(w1b[:], w1f[:])
        att1s_bc = bcast_row(
            as1_d.ap().rearrange("(o h) d -> o (h d)", o=1), OUT1, "a1s")
        att1d_bc = bcast_row(
            ad1_d.ap().rearrange("(o h) d -> o (h d)", o=1), OUT1, "a1d")

        # wfold[:, 0:8] = per-head fold of att_src1 into W1 columns;
        # wfold[:, 8:16] same for att_dst1:  a_src1(n) = x(n) @ wfold[:,0:8]
        wfold = const.tile([P, 2 * H1], bf16)
        for ci, att_bc in ((0, att1s_bc), (1, att1d_bc)):
            tmp = sb.tile([P, OUT1], f32, tag="tmpw2")
            nc.vector.tensor_tensor(tmp[:], w1f[:], att_bc[:], op=ALU.mult)
            red = sb.tile([P, H1], f32, tag="tmpw3")
            nc.vector.tensor_reduce(
                red[:], tmp[:].rearrange("p (h d) -> p h d", h=H1),
                axis=AX.X, op=ALU.add)
            nc.vector.tensor_copy(wfold[:, ci * H1:(ci + 1) * H1], red[:])

        adst_sb = const.tile([P, NT, H1], bf16)    # a_dst1 per local node
        adn2_sb = const.tile([P, NT, 1], bf16)     # a_dst2 per local node
        logits = const.tile([P, NT], f32, tag="logits")

        # ============ phase 0: build layer-1 table rows for local slice ====
        XB = 4   # x tiles per load
        for k in range(NT):
            r0 = k * P
            if k % XB == 0:
                kn = min(XB, NT - k)
                xf = sb3.tile([P, XB, IN], f32, tag="xf")
                nc.sync.dma_start(
                    xf[:, 0:kn, :],
                    x_sl.ap()[r0:r0 + kn * P, :].rearrange(
                        "(t p) f -> p t f", p=P))
            xb = sb3.tile([P, IN], bf16, tag="xb")
            nc.vector.tensor_copy(xb[:], xf[:, k % XB, :])
            xt_p = psB.tile([P, P], bf16, tag="tp")
            nc.tensor.transpose(xt_p[:], xb[:], idbf[:])
            xt = sb3.tile([P, P], bf16, tag="xt")
            nc.scalar.copy(xt[:], xt_p[:])
            h1_p = psA.tile([P, OUT1], f32, tag="zbig")
            nc.tensor.matmul(h1_p[:], lhsT=xt[:], rhs=w1b[:], start=True,
                             stop=True)
            fold_p = psA.tile([P, 2 * H1], f32, tag="ae")
            nc.tensor.matmul(fold_p[:], lhsT=xt[:], rhs=wfold[:], start=True,
                             stop=True)
            trow = sb.tile([P, TW1], bf16, tag="trow", bufs=B3)
            nc.vector.tensor_copy(trow[:, 0:OUT1], h1_p[:])
            nc.scalar.copy(trow[:, OUT1:OUT1 + H1], fold_p[:, 0:H1])
            nc.vector.tensor_copy(adst_sb[:, k, :], fold_p[:, H1:2 * H1])
            if k < KSPLIT:
                nc.scalar.dma_start(h1tab_inA.ap()[r0:r0 + P, :], trow[:])
            else:
                nc.scalar.dma_start(
                    h1tab_inB.ap()[r0 - ASL:r0 - ASL + P, :], trow[:])

        # late constants: loaded/broadcast while the table build drains
        idx16_sb = const.tile([P, TI // 16], i16)
        nc.sync.dma_start(idx16_sb[:], idx_d.ap())
        mask_sb = const.tile([P, NT], f32)
        nc.sync.dma_start(mask_sb[:], mask_d.ap())
        w2b = const.tile([P, W2C, HID], bf16)
        w2f_t = sb.tile([P, W2C, HID], f32, tag="tmpw")
        nc.sync.dma_start(
            w2f_t[:], W2_d.ap().rearrange("(c p) n -> p c n", p=P))
        nc.vector.tensor_copy(w2b[:], w2f_t[:])
        att2s_bc = bcast_row(as2_d.ap(), HID, "a2s")
        att2d_bc = bcast_row(ad2_d.ap(), HID, "a2d")
        b1_bc = bcast_row(b1_d.ap()[None, :], OUT1, "b1")
        b2_bc = bcast_row(b2_d.ap()[None, :], HID, "b2")
        fcb_bc = bcast_row(fcb_d.ap()[None, :], 1, "fcb")
        fcw_f = sb.tile([HID, 1], f32, tag="tmpw4")
        nc.sync.dma_start(fcw_f[:], fcw_d.ap())
        fcw_sb = const.tile([HID, 1], bf16)
        nc.vector.tensor_copy(fcw_sb[:], fcw_f[:])

        if profile:
            nc.sync.dma_start(h1tab.ap()[0:ASL, :], h1tab_inA.ap())
            nc.sync.dma_start(h1tab.ap()[HALF:HALF + NPCP - ASL, :],
                              h1tab_inB.ap())
        else:
            nc.gpsimd.collective_compute(
                "AllGather", ALU.bypass, replica_groups=RG,
                ins=[h1tab_inA.ap().opt()],
                outs=[h1tab.ap()[0:HALF, :].opt()])
            nc.gpsimd.collective_compute(
                "AllGather", ALU.bypass, replica_groups=RG,
                ins=[h1tab_inB.ap().opt()],
                outs=[h1tab.ap()[HALF:NTAB, :].opt()])

        # ================= shared edge-phase machinery =====================
        def edge_phase(layer):
            L1 = layer == 1
            table = h1tab if L1 else h2tab
            FW = TW1 if L1 else TW2    # table row cols
            NH = H1 if L1 else 1       # heads
            ZC = OUT1 if L1 else HID + 1   # z matmul cols
            lo_ap = table.ap()[0:HALF, :]
            hi_ap = table.ap()[HALF:NTAB, :]
            sfx = "1" if L1 else "2"
            GCHL = GCH if L1 else GCH2
            goff = 0
            for t in range(NT):
                Lt, Ht = sched[t]
                Kt = Lt + Ht
                gbase = t * P

                # --- one-hot S/ST ship + gathers ---
                sst_sb = sb.tile([P, Kt, 2, P], fp8, tag="sst" + sfx)
                sst_eng = nc.scalar if L1 else nc.sync
                sst_eng.dma_start(sst_sb[:], sst_d.ap()[:, goff:goff + Kt])
                X_all = sb.tile([P, Kt, FW], bf16, tag="X" + sfx)
                off16 = goff * P // 16
                for ci, (g0, gn, half_ap) in enumerate(
                        [(q, min(GCHL, Lt - q), lo_ap)
                         for q in range(0, Lt, GCHL)]
                        + [(Lt + q, min(GCHL, Ht - q), hi_ap)
                           for q in range(0, Ht, GCHL)]):
                    n = gn * P
                    idxs = idx16_sb[:, off16 + g0 * P // 16:
                                    off16 + (g0 * P + n) // 16]
                    nc.gpsimd.dma_gather(
                        X_all[:, g0:g0 + gn, :], half_ap, idxs, n, n, FW,
                        transpose=False)

                # --- per-edge logits for the whole tile (in one PSUM strip):
                # a_dst via one small matmul per group, then the gathered
                # a_src accumulated on top via an identity-lhsT matmul.
                ae_p = psA.tile([P, Kt, NH], f32, tag="ae")
                adn = (adst_sb if L1 else adn2_sb)[:, t, :]
                for j in range(Kt):
                    nc.tensor.matmul(ae_p[:, j, :], lhsT=sst_sb[:, j, 1, :],
                                     rhs=adn, start=(j == 0), stop=False)
                asrc = (X_all[:, :, OUT1:OUT1 + H1] if L1
                        else X_all[:, :, HID + 1:HID + 2])
                nc.tensor.matmul(ae_p[:], lhsT=idbf[:], rhs=asrc,
                                 start=False, stop=True)
                ae_sb = sb.tile([P, Kt, NH], f32, tag="aesb" + sfx, bufs=B3)
                nc.scalar.copy(ae_sb[:].rearrange("p k h -> p (k h)"),
                               ae_p[:].rearrange("p k h -> p (k h)"))
                lr = sb.tile([P, Kt, NH], f32, tag="lr" + sfx, bufs=B3)
                nc.vector.scalar_tensor_tensor(
                    lr[:], in0=ae_sb[:], scalar=0.2, in1=ae_sb[:],
                    op0=ALU.mult, op1=ALU.max)
                p_all = sb.tile([P, Kt, NH], bf16, tag="p" + sfx, bufs=B4)
                nc.scalar.activation(
                    p_all[:].rearrange("p k h -> p (k h)"),
                    lr[:].rearrange("p k h -> p (k h)"), ACTF.Exp)

                # --- segment sums via one-hot matmuls ---
                z_p = psA.tile([P, ZC], f32, tag="zbig")
                if L1:
                    s_p = psA.tile([P, H1], f32, tag="ae")
                    pdup = sb.tile([P, Kt, H1, 2], bf16, tag="pdup", bufs=B3)
                    nc.vector.tensor_copy(
                        pdup[:],
                        p_all[:, :, :, None].to_broadcast([P, Kt, H1, 2]))
                for m0 in range(0, Kt, MCH):
                    mk = min(MCH, Kt - m0)
                    if L1:
                        M_c = sb.tile([P, MCH, H1, 32, 2], bf16, tag="M", bufs=B3)
                        nc.vector.tensor_tensor(
                            M_c[:, 0:mk],
                            X_all[:, m0:m0 + mk, 0:OUT1].rearrange(
                                "p k (h f g) -> p k h f g", h=H1, g=2),
                            pdup[:, m0:m0 + mk, :, None, :].to_broadcast(
                                [P, mk, H1, 32, 2]),
                            op=ALU.mult)
                    else:
                        M_c = sb.tile([P, MCH, ZC], bf16, tag="M2", bufs=B3)
                        nc.vector.tensor_tensor(
                            M_c[:, 0:mk],
                            X_all[:, m0:m0 + mk, 0:ZC],
                            p_all[:, m0:m0 + mk, :].to_broadcast([P, mk, ZC]),
                            op=ALU.mult)
                    for j in range(m0, m0 + mk):
                        rhs = (M_c[:, j - m0].rearrange("p h f g -> p (h f g)")
                               if L1 else M_c[:, j - m0])
                        nc.tensor.matmul(z_p[:], lhsT=sst_sb[:, j, 0, :],
                                         rhs=rhs, start=(j == 0),
                                         stop=(j == Kt - 1))
                        if L1:
                            nc.tensor.matmul(s_p[:], lhsT=sst_sb[:, j, 0, :],
                                             rhs=p_all[:, j, :],
                                             start=(j == 0),
                                             stop=(j == Kt - 1))
                goff += Kt

                # ---------------- finalize dst tile ----------------
                if L1:
                    s_eps = sb.tile([P, H1], f32, tag="seps")
                    nc.vector.tensor_scalar(s_eps[:], s_p[:], 1e-16, None,
                                            op0=ALU.add)
                    s_inv = sb.tile([P, H1], f32, tag="sinv", bufs=B3)
                    nc.vector.reciprocal(s_inv[:], s_eps[:])
                    y = sb.tile([P, OUT1], f32, tag="y")
                    nc.vector.tensor_tensor(
                        y[:].rearrange("p (h f) -> p h f", h=H1),
                        z_p[:].rearrange("p (h f) -> p h f", h=H1),
                        s_inv[:, :, None].to_broadcast([P, H1, HID]),
                        op=ALU.mult)
                    WE = OUT1
                    yb = sb.tile([P, WE], f32, tag="yb")
                    nc.vector.tensor_tensor(yb[:], y[:], b1_bc[:], op=ALU.add)
                else:
                    s2e = sb.tile([P, 1], f32, tag="s2e")
                    nc.vector.tensor_scalar(s2e[:], z_p[:, HID:HID + 1],
                                            1e-16, None, op0=ALU.add)
                    s2i = sb.tile([P, 1], f32, tag="s2i")
                    nc.vector.reciprocal(s2i[:], s2e[:])
                    y = sb.tile([P, HID], f32, tag="y2")
                    nc.vector.tensor_scalar(y[:], z_p[:, 0:HID], s2i[:, 0:1],
                                            None, op0=ALU.mult)
                    WE = HID
                    yb = sb.tile([P, WE], f32, tag="yb2")
                    nc.vector.tensor_tensor(yb[:], y[:], b2_bc[:], op=ALU.add)
                # elu(yb) = relu(yb) + exp(min(yb,0)) - 1
                t0 = sb.tile([P, WE], f32, tag="elu0" + sfx)
                nc.vector.tensor_scalar_min(t0[:], yb[:], 0.0)
                ex = sb.tile([P, WE], f32, tag="elu1" + sfx)
                nc.scalar.activation(ex[:], t0[:], ACTF.Exp)
                ry = sb.tile([P, WE], f32, tag="elu2" + sfx)
                nc.scalar.activation(ry[:], yb[:], ACTF.Relu)
                x2 = sb.tile([P, WE], bf16, tag="x2" + sfx, bufs=B3)
                nc.vector.scalar_tensor_tensor(
                    x2[:], in0=ex[:], scalar=-1.0, in1=ry[:],
                    op0=ALU.add, op1=ALU.add)

                if L1:
                    # h2 = x2 @ W2, then pack layer-2 table row
                    h2_p = psA.tile([P, HID], f32, tag="zbig")
                    for cix in range(W2C):
                        xt_p = psB.tile([P, P], bf16, tag="tp")
                        nc.tensor.transpose(
                            xt_p[:], x2[:, cix * P:(cix + 1) * P], idbf[:])
                        xt2 = sb3.tile([P, P], bf16, tag="xt2")
                        nc.scalar.copy(xt2[:], xt_p[:])
                        nc.tensor.matmul(h2_p[:], lhsT=xt2[:],
                                         rhs=w2b[:, cix, :],
                                         start=(cix == 0),
                                         stop=(cix == W2C - 1))
                    trow2 = sb.tile([P, TW2], bf16, tag="trow2", bufs=B3)
                    nc.scalar.copy(trow2[:, 0:HID], h2_p[:])
                    nc.vector.memset(trow2[:, HID:HID + 1], 1.0)
                    tmp = sb.tile([P, HID], f32, tag="atmp")
                    ared = sb.tile([P, 1], f32, tag="ared")
                    nc.vector.tensor_tensor(tmp[:], h2_p[:], att2s_bc[:],
                                            op=ALU.mult)
                    nc.vector.tensor_reduce(ared[:], tmp[:], axis=AX.X,
                                            op=ALU.add)
                    nc.vector.tensor_copy(trow2[:, HID + 1:HID + 2], ared[:])
                    ared2 = sb.tile([P, 1], f32, tag="ared2")
                    nc.vector.tensor_tensor(tmp[:], h2_p[:], att2d_bc[:],
                                            op=ALU.mult)
                    nc.vector.tensor_reduce(ared2[:], tmp[:], axis=AX.X,
                                            op=ALU.add)
                    nc.vector.tensor_copy(adn2_sb[:, t, :], ared2[:])
                    if t < KSPLIT:
                        nc.scalar.dma_start(
                            h2tab_inA.ap()[gbase:gbase + P, :], trow2[:])
                    else:
                        nc.scalar.dma_start(
                            h2tab_inB.ap()[gbase - ASL:gbase - ASL + P, :],
                            trow2[:])
                else:
                    x2t_p = psB.tile([HID, P], bf16, tag="tp")
                    nc.tensor.transpose(x2t_p[:], x2[:, 0:HID], idbf[:])
                    x2t = sb3.tile([HID, P], bf16, tag="x2t")
                    nc.scalar.copy(x2t[:], x2t_p[:])
                    lg_p = psB.tile([P, 1], f32, tag="tp")
                    nc.tensor.matmul(lg_p[:], lhsT=x2t[:], rhs=fcw_sb[:],
                                     start=True, stop=True)
                    nc.scalar.activation(logits[:, t:t + 1],
                                         lg_p[:], ACTF.Identity,
                                         bias=fcb_bc[:])

        # ================= layer 1 =================
        edge_phase(1)
        if profile:
            nc.sync.dma_start(h2tab.ap()[0:ASL, :], h2tab_inA.ap())
            nc.sync.dma_start(h2tab.ap()[HALF:HALF + NPCP - ASL, :],
                              h2tab_inB.ap())
        else:
            nc.gpsimd.collective_compute(
                "AllGather", ALU.bypass, replica_groups=RG,
                ins=[h2tab_inA.ap().opt()],
                outs=[h2tab.ap()[0:HALF, :].opt()])
            nc.gpsimd.collective_compute(
                "AllGather", ALU.bypass, replica_groups=RG,
                ins=[h2tab_inB.ap().opt()],
                outs=[h2tab.ap()[HALF:NTAB, :].opt()])

        # ================= layer 2 =================
        edge_phase(2)

        # ================= softmax over all nodes =================
        lgm = sb.tile([P, NT], f32, tag="lgm")
        nc.vector.tensor_tensor(lgm[:], logits[:], mask_sb[:], op=ALU.add)
        ex_all = sb.tile([P, NT], f32, tag="exall")
        nc.scalar.activation(ex_all[:], lgm[:], ACTF.Exp)
        part = sb.tile([P, 1], f32, tag="part")
        nc.vector.tensor_reduce(part[:], ex_all[:], axis=AX.X, op=ALU.add)
        tot_p = psB.tile([1, 1], f32, tag="tp")
        nc.tensor.matmul(tot_p[:], lhsT=part[:], rhs=ones_c[:], start=True,
                         stop=True)
        tot_sb = sb.tile([1, 1], f32, tag="tot")
        nc.scalar.copy(tot_sb[:], tot_p[:])
        nc.sync.dma_start(ssum_in.ap(), tot_sb[:])
        if profile:
            nc.sync.dma_start(ssum.ap(), ssum_in.ap())
        else:
            nc.gpsimd.collective_compute(
                "AllReduce", ALU.add, replica_groups=RG,
                ins=[ssum_in.ap().opt()], outs=[ssum.ap().opt()])
        gsum = sb.tile([1, 1], f32, tag="gsum")
        nc.sync.dma_start(gsum[:], ssum.ap())
        ginv = sb.tile([1, 1], f32, tag="ginv")
        nc.vector.reciprocal(ginv[:], gsum[:])
        ginv_p = psB.tile([P, 1], f32, tag="tp")
        nc.tensor.matmul(ginv_p[:], lhsT=ones_r[:], rhs=ginv[:], start=True,
                         stop=True)
        ginv_bc = sb.tile([P, 1], f32, tag="ginvbc")
        nc.scalar.copy(ginv_bc[:], ginv_p[:])
        res = sb.tile([P, NT], f32, tag="res")
        nc.vector.tensor_scalar(res[:], ex_all[:], ginv_bc[:], None,
                                op0=ALU.mult)
        nc.sync.dma_start(
            out_d.ap().rearrange("(t p) o -> p (t o)", p=P), res[:])

    nc.compile()
    return nc


# ----------------------------------------------------------------------------
# entry point
# ----------------------------------------------------------------------------
def build_in_maps(inputs, cfg):
    sched, idx16, sst, mask, core_tiles, perm, TI, HALF = prep_edges(
        np.asarray(inputs["edge_index"]), cfg)
    x = np.asarray(inputs["x"], dtype=np.float32)
    N, NC = cfg["N"], cfg["NCORES"]
    NPCP = NT * P
    common = {k: np.ascontiguousarray(np.asarray(inputs[k], np.float32))
              for k in ("W1", "att_src1", "att_dst1", "b1", "W2", "att_src2",
                        "att_dst2", "b2", "fc_w", "fc_b")}
    xperm = np.zeros((NC * NPCP, x.shape[1]), dtype=np.float32)
    xperm[perm] = x
    in_maps = []
    for c in range(NC):
        m = dict(common)
        m["x_tiles"] = np.ascontiguousarray(xperm[c * NPCP:(c + 1) * NPCP])
        m["idx16"] = np.ascontiguousarray(idx16[c])
        m["sst"] = np.ascontiguousarray(sst[c])
        m["padmask"] = np.ascontiguousarray(mask[c])
        in_maps.append(m)
    return in_maps, sched, TI, HALF, perm


def kernel(**inputs) -> np.ndarray:
    from concourse import bass_utils

    cfg = dict(CFG)
    in_maps, sched, TI, HALF, perm = build_in_maps(inputs, cfg)
    nc = build_kernel(cfg, sched, TI, HALF)
    res = bass_utils.run_bass_kernel_spmd(
        nc, in_maps, core_ids=list(range(cfg["NCORES"])),
        trace=bool(int(os.environ.get("GAT_TRACE", "0"))))
    kernel.last_results = res
    allout = np.concatenate([r["out"] for r in res.results], axis=0)
    return allout[perm].astype(np.float32)
